# revision 16
# baseline (speedup 1.0000x reference)
"""Sliding-window causal GQA attention block (QKV proj + RoPE + SDPA + out proj)
on 8 Trainium2 NeuronCores.

Sharding: 8 cores = 2 batches x 4 sequence chunks of 512 tokens. Each core
computes the full attention-block output for its (batch, seq-chunk):
  - Q projection for its 512 queries (all 16 heads) in transposed [d, s] layout
  - K/V projection for its chunk + 512-token halo (sliding window support)
  - RoPE via rotate-half permutation matmul + element-wise mul/add
  - sliding-window causal attention with scores kept transposed [keys, queries]
  - softmax denominators via 3 ones/valid-vector matmuls over pair-summed
    probability tiles (interior key tiles are provably unmasked, so their
    exp tiles feed AV directly and the per-core j>=0 validity is a
    per-partition scalar folded into the denominator matmul)
  - out-projection computed transposed (y^T = wo^T-tiles @ o^T) in two passes
    (heads 0-11 overlapped with the last attention group, heads 12-15 after)

Pipelining: attention for kv-group i is interleaved instruction-by-
instruction with the Q projection of group i+1 (and the last group with the
out-projection A pass) so the tensor engine never waits on softmax exp.

Matmul operands are bf16, accumulation fp32 in PSUM.
"""
import numpy as np

import concourse.bacc as bacc
import concourse.mybir as mybir
import concourse.tile as tile
from concourse.bass_utils import run_bass_kernel_spmd

# Problem constants (hardcoded per contract)
B, S, E = 2, 2048, 2048
H, KV, D = 16, 4, 128
WIN = 512
THETA = 1e6
NCORES = 8
CH = 512          # seq chunk per core
SW = 1024         # K/V window per core (halo 512 + own 512)
P = 128
ECH = E // P      # 16 contraction chunks
NJT = SW // P     # 8 key tiles in window
F32 = mybir.dt.float32
BF16 = mybir.dt.bfloat16
SCALE = 1.0 / float(np.sqrt(np.float32(D)))
EDGE_R = (0, 1, 4, 5)   # key tiles needing a full mask; r=2,3 are interior

_CACHE = {}


def _build():
    nc = bacc.Bacc("TRN2", target_bir_lowering=False, debug=False,
                   num_devices=NCORES)

    xt = nc.dram_tensor("xt", [E, CH], BF16, kind="ExternalInput")
    wqkv = nc.dram_tensor("wqkv", [E, (H + 2 * KV) * D], BF16, kind="ExternalInput")
    wo = nc.dram_tensor("wo", [H * D, E], BF16, kind="ExternalInput")
    cosw = nc.dram_tensor("cosw", [P, CH], BF16, kind="ExternalInput")
    sinw = nc.dram_tensor("sinw", [P, CH], BF16, kind="ExternalInput")
    masks = nc.dram_tensor("masks", [8, P, CH], BF16, kind="ExternalInput")
    perm = nc.dram_tensor("perm", [P, P], BF16, kind="ExternalInput")
    ident = nc.dram_tensor("ident", [P, P], BF16, kind="ExternalInput")
    vm = nc.dram_tensor("vm", [2, P, P], BF16, kind="ExternalInput")
    yt = nc.dram_tensor("yt", [E, CH], BF16, kind="ExternalOutput")

    KOFF = H * D            # w_qkv column offsets
    VOFF = H * D + KV * D

    with tile.TileContext(nc) as tc:
        with (
            tc.tile_pool(name="res", bufs=1) as res,       # resident tensors
            tc.tile_pool(name="big", bufs=2) as big,       # wv -> o_T -> yA
            tc.tile_pool(name="wst", bufs=4) as wst,       # streamed wk/wq tiles
            tc.tile_pool(name="wop", bufs=8) as wop,       # streamed wo tiles
            tc.tile_pool(name="tmp", bufs=3) as tmp,       # transient compute
            tc.tile_pool(name="att", bufs=6) as att,       # pe/pt (6 live/blk)
            tc.tile_pool(name="pj", bufs=2, space="PSUM") as pj,
            tc.tile_pool(name="ps1", bufs=3, space="PSUM") as ps1,  # scores
            tc.tile_pool(name="ps2", bufs=2, space="PSUM") as ps2,  # av
            tc.tile_pool(name="psd", bufs=1, space="PSUM") as psd,  # denom
            tc.tile_pool(name="dram", bufs=1, space="DRAM") as dram,
        ):
            # ---- DMA layout: sync queue = x halo + streamed wq/wo/yt;
            # gpsimd queue = wk, rope constants, x own, wv, masks (ordered
            # just-in-time so the first K chain starts ~10us in). ------------
            wqkv3 = wqkv.ap().rearrange("(eo p) f -> p eo f", p=P)
            wk_t = {}

            def load_wk(fk):
                wk_t[fk] = wst.tile([P, ECH, P], BF16, tag="wqk",
                                    name=f"wk_{fk}")
                for eh in range(2):
                    sl = slice(eh * 8, eh * 8 + 8)
                    nc.gpsimd.dma_start(
                        wk_t[fk][:, sl, :],
                        wqkv3[:, sl, KOFF + fk * P:KOFF + (fk + 1) * P])

            xt3 = xt.ap().rearrange("(eo p) s -> p eo s", p=P)
            xo = []
            load_wk(0)
            cos_sb = res.tile([P, CH], BF16, tag="cosw")
            sin_sb = res.tile([P, CH], BF16, tag="sinw")
            nc.gpsimd.dma_start(cos_sb[:], cosw.ap())
            nc.gpsimd.dma_start(sin_sb[:], sinw.ap())
            perm_sb = res.tile([P, P], BF16, tag="perm")
            nc.gpsimd.dma_start(perm_sb[:], perm.ap())
            vm_sb = res.tile([P, 2, P], BF16, tag="vm")
            for vi in range(2):
                nc.gpsimd.dma_start(vm_sb[:, vi, :], vm.ap()[vi])
            load_wk(1)
            for i in range(4):
                t = res.tile([P, 4, CH], BF16, tag=f"xo{i}")
                nc.sync.dma_start(t[:], xt3[:, 4 * i:4 * i + 4, 0:CH])
                xo.append(t)
                if i == 1:
                    load_wk(2)
            load_wk(3)
            ident_sb = res.tile([P, P], BF16, tag="ident")
            nc.gpsimd.dma_start(ident_sb[:], ident.ap())
            # zero tile for the left-edge halo slot of the exchanges
            zkv_sb = res.tile([P, 4, CH], BF16, tag="zkv")
            nc.vector.memset(zkv_sb[:], 0.0)
            # wv resident in one big-pool slot, [p, e_chunk, v_cols 512]
            wv_sb = big.tile([P, ECH, KV * D], BF16, tag="big")
            for eh in range(4):
                sl = slice(eh * 4, eh * 4 + 4)
                nc.sync.dma_start(wv_sb[:, sl, :],
                                  wqkv3[:, sl, VOFF:VOFF + KV * D])
            mask_sb = res.tile([P, 8, CH], BF16, tag="masks")
            for mi in range(8):
                nc.sync.dma_start(mask_sb[:, mi, :], masks.ap()[mi])

            def xos(e):
                return xo[e // 4][:, e % 4, :]

            def x_win_slice(e, st):
                """lhsT [128 e-part, 128 s-cols] for own s-tile st (4..7)."""
                return xos(e)[:, (st - 4) * P:(st - 3) * P]

            # -------- rope helper, software-pipelined by one chain ----------
            # rope() emits only the scalar PSUM->SBUF cast now; the perm
            # matmul + muls are deferred (flush_rope) until after the next
            # matmul chain so the tensor engine never waits on the cast.
            pending_rope = [None]

            def _rope_finish(dst, raw_sb, c0, c1, split2):
                n = c1 - c0
                rot_ps = ps1.tile([P, CH], F32, tag="sc")
                nc.tensor.matmul(rot_ps[:, :n], perm_sb[:], raw_sb[:, :n],
                                 start=True, stop=True)
                t1 = tmp.tile([P, CH], F32, tag="t1")
                nc.gpsimd.tensor_mul(out=t1[:, :n], in0=raw_sb[:, :n],
                                     in1=cos_sb[:, c0:c1])
                t2 = tmp.tile([P, CH], F32, tag="t2")
                nc.vector.tensor_mul(out=t2[:, :n], in0=rot_ps[:, :n],
                                     in1=sin_sb[:, c0:c1])
                if split2:
                    nc.gpsimd.tensor_add(
                        out=dst,
                        in0=t1[:, :n].rearrange("p (a b) -> p a b", a=2),
                        in1=t2[:, :n].rearrange("p (a b) -> p a b", a=2))
                else:
                    nc.gpsimd.tensor_add(out=dst, in0=t1[:, :n], in1=t2[:, :n])

            def rope(dst, raw_ps, c0, c1, split2=False):
                n = c1 - c0
                raw_sb = att.tile([P, CH], BF16, tag="qraw")
                nc.scalar.copy(out=raw_sb[:, :n], in_=raw_ps[:, :n])
                pending_rope[0] = (dst, raw_sb, c0, c1, split2)

            def flush_rope():
                if pending_rope[0] is not None:
                    _rope_finish(*pending_rope[0])
                    pending_rope[0] = None

            # ------- K projection, own tokens only (halo via exchange) ------
            k_sb = res.tile([P, KV, SW], BF16, tag="k")
            for fk in range(KV):
                k_ps = pj.tile([P, CH], F32, tag="pj")
                for e in range(ECH):
                    nc.tensor.matmul(k_ps[:], wk_t[fk][:, e, :], xos(e),
                                     start=(e == 0), stop=(e == ECH - 1))
                flush_rope()
                rope(k_sb[:, fk, CH:SW], k_ps, 0, CH)

            # ------------- Q projection (transposed [d, s] layout) -----------
            # q_sb free layout: block blk = kv*4 + hp*2 + p2 (16 blocks of 512);
            # within a block: [head-sub 0 | head-sub 1] x 256 queries.
            q_sb = res.tile([P, 16, CH], BF16, tag="q")

            def emit_q_dma(fi):
                wq_t = wst.tile([P, ECH, P], BF16, tag="wqk", name=f"wq_{fi}")
                for eh in range(2):
                    sl = slice(eh * 8, eh * 8 + 8)
                    nc.sync.dma_start(wq_t[:, sl, :],
                                      wqkv3[:, sl, fi * P:(fi + 1) * P])
                return wq_t

            def emit_q_chain(fi, wq_t, e0, e1, q_ps):
                for e in range(e0, e1):
                    nc.tensor.matmul(q_ps[:], wq_t[:, e, :], xos(e),
                                     start=(e == 0), stop=(e == ECH - 1))

            def emit_q_rope(fi, q_ps):
                kvb, hp, sub = fi // 4, (fi % 4) // 2, fi % 2
                blk0 = kvb * 4 + hp * 2
                dst = q_sb[:, blk0:blk0 + 2, sub * 256:sub * 256 + 256]
                rope(dst, q_ps, 0, CH, split2=True)

            for fi in range(4):         # kv-group 0 heads, un-interleaved
                wq_t = emit_q_dma(fi)
                q_ps = pj.tile([P, CH], F32, tag="pj")
                emit_q_chain(fi, wq_t, 0, ECH, q_ps)
                flush_rope()
                emit_q_rope(fi, q_ps)

            # ---- K halo exchange: AllGather own K across the batch group ----
            GROUPS = [[0, 1, 2, 3], [4, 5, 6, 7]]
            inbK = dram.tile([P, 4, CH], BF16, name="inbK")
            allK = dram.tile([5, P, 4, CH], BF16, name="allK")
            nc.gpsimd.dma_start(allK[0], zkv_sb[:])
            nc.gpsimd.dma_start(inbK[:], k_sb[:, :, CH:SW])
            nc.gpsimd.collective_compute(
                "AllGather", mybir.AluOpType.bypass,
                replica_groups=GROUPS,
                ins=[inbK.opt()], outs=[allK[1:5].opt()])

            # ------------- V projection (natural [s, d] layout) --------------
            v_sb = res.tile([P, NJT, KV * D], BF16, tag="v")
            for st in range(4, NJT):
                v_ps = pj.tile([P, KV * D], F32, tag="pj")
                for e in range(ECH):
                    nc.tensor.matmul(v_ps[:], x_win_slice(e, st), wv_sb[:, e, :],
                                     start=(e == 0), stop=(e == ECH - 1))
                flush_rope()
                nc.scalar.copy(out=v_sb[:, st, :], in_=v_ps[:])

            # ---- V halo exchange ----
            inbV = dram.tile([P, 4, KV * D], BF16, name="inbV")
            allV = dram.tile([5, P, 4, KV * D], BF16, name="allV")
            nc.gpsimd.dma_start(allV[0], zkv_sb[:])
            nc.gpsimd.dma_start(inbV[:], v_sb[:, 4:8, :])
            nc.gpsimd.collective_compute(
                "AllGather", mybir.AluOpType.bypass,
                replica_groups=GROUPS,
                ins=[inbV.opt()], outs=[allV[1:5].opt()])
            rank = nc.gpsimd.cc_rank(GROUPS)
            nc.gpsimd.dma_start(k_sb[:, :, 0:CH], allK[rank])
            nc.gpsimd.dma_start(v_sb[:, 0:4, :], allV[rank])

            # ---- attention + interleaved fillers ---------------------------
            # blk (kvb, hp, p2) covers queries [p2*256, p2*256+256) of heads
            # 4kvb+2hp and 4kvb+2hp+1 (512 score columns), key tiles
            # jt = 2*p2 + r for r in 0..5.  r in {2,3} is interior: fully
            # inside the sliding window, so no mask multiply; per-core j>=0
            # validity enters via valid_sb in the denominator matmul.
            o_sb = big.tile([P, 16, CH], BF16, tag="big")
            yA_sb = big.tile([P, 16, CH], BF16, tag="big")
            wo3 = wo.ap().rearrange("(fo p) e2 -> p fo e2", p=P)

            def attn_blk(kvb, hp, p2, fillers):
                """Emit one attention block; fillers = (f1, f2) callables
                emitted between score groups to cover softmax latency."""
                blk = kvb * 4 + hp * 2 + p2
                sc = []
                pe = []
                pt = {}
                acc = {}

                def sc_group(rr):
                    for r in rr:
                        jt = 2 * p2 + r
                        sc_ps = ps1.tile([P, CH], F32, tag="sc",
                                         name=f"sc_{blk}_{r}")
                        nc.tensor.matmul(sc_ps[:],
                                         k_sb[:, kvb, jt * P:(jt + 1) * P],
                                         q_sb[:, blk, :],
                                         start=True, stop=True)
                        sc.append(sc_ps)
                        pe_t = att.tile([P, CH], BF16, tag="pe")
                        nc.scalar.activation(
                            out=pe_t[:], in_=sc_ps[:],
                            func=mybir.ActivationFunctionType.Exp,
                            scale=SCALE)
                        pe.append(pe_t)
                        if r in EDGE_R:
                            ei = EDGE_R.index(r)
                            pt_t = att.tile([P, CH], BF16, tag="pt")
                            nc.vector.tensor_mul(
                                out=pt_t[:], in0=pe_t[:],
                                in1=mask_sb[:, p2 * 4 + ei, :])
                            pt[r] = pt_t
                        if r == 1:
                            a = tmp.tile([P, CH], BF16, tag="acc")
                            nc.vector.tensor_add(out=a[:], in0=pt[0][:],
                                                 in1=pt[1][:])
                            acc["01"] = a
                        if r == 3:
                            a = tmp.tile([P, CH], BF16, tag="acc")
                            nc.vector.tensor_add(out=a[:], in0=pe[2][:],
                                                 in1=pe[3][:])
                            acc["23"] = a
                        if r == 5:
                            a = tmp.tile([P, CH], BF16, tag="acc")
                            nc.vector.tensor_add(out=a[:], in0=pt[4][:],
                                                 in1=pt[5][:])
                            acc["45"] = a

                sc_group((0, 1, 2))
                fillers[0]()
                sc_group((3, 4, 5))
                fillers[1]()

                av_ps = ps2.tile([P, CH], F32, tag="av")
                for r in range(6):
                    jt = 2 * p2 + r
                    rhs = pt[r] if r in EDGE_R else pe[r]
                    nc.tensor.matmul(av_ps[:],
                                     v_sb[:, jt, kvb * D:(kvb + 1) * D],
                                     rhs[:], start=(r == 0), stop=(r == 5))
                vi = 1 if p2 == 0 else 0
                dn_ps = psd.tile([P, CH], F32, tag="dn")
                nc.tensor.matmul(dn_ps[:], vm_sb[:, 0, :], acc["01"][:],
                                 start=True, stop=False)
                nc.tensor.matmul(dn_ps[:], vm_sb[:, vi, :], acc["23"][:],
                                 start=False, stop=False)
                nc.tensor.matmul(dn_ps[:], vm_sb[:, 0, :], acc["45"][:],
                                 start=False, stop=True)
                rc = tmp.tile([P, CH], F32, tag="bc")
                nc.vector.reciprocal_approx_fast(out=rc[:], in_=dn_ps[:])
                nc.vector.tensor_mul(out=o_sb[:, blk, :],
                                     in0=av_ps[:], in1=rc[:])

            def emit_outA_dma(et):
                wo_t = wop.tile([P, 12, P], BF16, tag="woA", name=f"woA_{et}")
                nc.sync.dma_start(wo_t[:, 0:8, :],
                                  wo3[:, 0:8, et * P:(et + 1) * P])
                nc.sync.dma_start(wo_t[:, 8:12, :],
                                  wo3[:, 8:12, et * P:(et + 1) * P])
                return wo_t

            def o_slice(f):
                kvb, hp, sub = f // 4, (f % 4) // 2, f % 2
                blk0 = kvb * 4 + hp * 2
                return o_sb[:, blk0:blk0 + 2, sub * 256:sub * 256 + 256]

            outA_dmas = {}

            def emit_outA_chain(et):
                if et not in outA_dmas:
                    outA_dmas[et] = emit_outA_dma(et)
                wo_t = outA_dmas[et]
                y_ps = pj.tile([P, CH], F32, tag="pj")
                for f in range(12):
                    nc.tensor.matmul(y_ps[:], wo_t[:, f, :], o_slice(f),
                                     start=(f == 0), stop=(f == 11))
                nc.vector.tensor_copy(out=yA_sb[:, et, :], in_=y_ps[:])

            # pairings 0..2: attention kvb with Q projection of kvb+1
            for kvb in range(3):
                fis = [4 * (kvb + 1) + j for j in range(4)]
                for b, (hp, p2) in enumerate(((0, 0), (0, 1), (1, 0), (1, 1))):
                    fi = fis[b]
                    wq_t = emit_q_dma(fi)
                    q_ps_box = []

                    def f1(fi=fi, wq_t=wq_t, box=q_ps_box):
                        flush_rope()
                        q_ps = pj.tile([P, CH], F32, tag="pj")
                        box.append(q_ps)
                        emit_q_chain(fi, wq_t, 0, 8, q_ps)

                    def f2(fi=fi, wq_t=wq_t, box=q_ps_box):
                        q_ps = box[0]
                        emit_q_chain(fi, wq_t, 8, ECH, q_ps)
                        emit_q_rope(fi, q_ps)

                    attn_blk(kvb, hp, p2, (f1, f2))

            # prefetch first out-proj A weight tiles during pairing 2 tail
            for et in range(6):
                outA_dmas[et] = emit_outA_dma(et)

            # pairing 3: attention kvb=3 with out-projection pass A (f 0..11)
            eti = [0]
            for b, (hp, p2) in enumerate(((0, 0), (0, 1), (1, 0), (1, 1))):
                def f1():
                    flush_rope()
                    emit_outA_chain(eti[0])
                    eti[0] += 1
                    emit_outA_chain(eti[0])
                    eti[0] += 1

                attn_blk(3, hp, p2, (f1, f1))

            # ------------- out projection pass B (f 12..15) + combine --------
            woB_dmas = {}

            def emit_outB_dma(et):
                wo_t = wop.tile([P, 4, P], BF16, tag="woB", name=f"woB_{et}")
                nc.sync.dma_start(wo_t[:], wo3[:, 12:16, et * P:(et + 1) * P])
                return wo_t

            for et in range(8):
                woB_dmas[et] = emit_outB_dma(et)
            for et in range(ECH):
                wo_t = woB_dmas.pop(et, None) or emit_outB_dma(et)
                if et + 8 < ECH:
                    woB_dmas[et + 8] = emit_outB_dma(et + 8)
                ypool, ytag = (pj, "pj") if et % 2 == 0 else (ps2, "av")
                y_ps = ypool.tile([P, CH], F32, tag=ytag, name=f"yB_{et}")
                for j, f in enumerate(range(12, 16)):
                    nc.tensor.matmul(y_ps[:], wo_t[:, j, :], o_slice(f),
                                     start=(j == 0), stop=False)
                nc.tensor.matmul(y_ps[:], ident_sb[:], yA_sb[:, et, :],
                                 start=False, stop=True)
                y_sb = tmp.tile([P, CH], BF16, tag="ysb")
                nc.scalar.copy(out=y_sb[:], in_=y_ps[:])
                nc.gpsimd.dma_start(yt.ap()[et * P:(et + 1) * P, :], y_sb[:])

    nc.compile()
    return nc


def _host_constants():
    inv_freq = (1.0 / (THETA ** (np.arange(0, D, 2, dtype=np.float32) / D))
                ).astype(np.float32)
    ang = np.arange(S, dtype=np.float32)[:, None] * inv_freq[None, :]
    emb = np.concatenate([ang, ang], axis=-1)          # [S, D]
    import ml_dtypes
    cos_t = np.ascontiguousarray(np.cos(emb).astype(np.float32).T
                                 ).astype(ml_dtypes.bfloat16)  # [D, S]
    sin_t = np.ascontiguousarray(np.sin(emb).astype(np.float32).T
                                 ).astype(ml_dtypes.bfloat16)
    pm = np.zeros((P, P), dtype=np.float32)            # rotate-half as lhsT
    a = np.arange(64)
    pm[a, a + 64] = 1.0
    pm[a + 64, a] = -1.0
    pm = pm.astype(ml_dtypes.bfloat16)

    identv = np.eye(P, dtype=np.float32).astype(ml_dtypes.bfloat16)
    return cos_t, sin_t, pm, identv


def _masks_for_chunk(chunk):
    """[8, 128, 512] bf16: edge masks m[p2*4+ei] for r in EDGE_R.

    Columns are [head-a 256 queries | head-b 256 queries] of pair p2; the
    mask depends only on the query position, so the two halves are equal."""
    import ml_dtypes
    m = np.zeros((8, P, CH), dtype=np.float32)
    s0 = chunk * CH
    for p2 in range(2):
        q_glob = s0 + p2 * 256 + np.arange(256)[None, :]
        for ei, r in enumerate(EDGE_R):
            jt = 2 * p2 + r
            jg0 = s0 - WIN + jt * P
            j_glob = jg0 + np.arange(P)[:, None]
            dlt = q_glob - j_glob
            ok = ((dlt >= 0) & (dlt < WIN) & (j_glob >= 0)).astype(np.float32)
            m[p2 * 4 + ei] = np.concatenate([ok, ok], axis=1)
    return m.astype(ml_dtypes.bfloat16)


def _vm_for_chunk(chunk):
    """[2, 128, 128] bf16 all-column-equal matmul masks for the denominator
    chain: vm[0] = all ones; vm[1][j, :] = 1 iff key row j of the p2=0
    interior tiles has j_glob >= 0 (whole tiles share validity since tile
    edges are 128-aligned, so this is 0s or 1s per chunk)."""
    import ml_dtypes
    s0 = chunk * CH
    v = np.ones((2, P, P), dtype=np.float32)
    if s0 - WIN + 2 * P < 0:
        v[1] = 0.0
    return v.astype(ml_dtypes.bfloat16)


def _prepare_in_maps(x, w_qkv, w_o):
    import ml_dtypes
    cos_t, sin_t, pm, identv = _host_constants()
    w_qkv = np.ascontiguousarray(w_qkv, dtype=np.float32).astype(ml_dtypes.bfloat16)
    w_o = np.ascontiguousarray(w_o, dtype=np.float32).astype(ml_dtypes.bfloat16)
    in_maps = []
    xts = [np.ascontiguousarray(np.asarray(x[b], dtype=np.float32).T
                                ).astype(ml_dtypes.bfloat16)
           for b in range(B)]
    for c in range(NCORES):
        b, chunk = divmod(c, 4)
        s0 = chunk * CH
        in_maps.append({
            "xt": np.ascontiguousarray(xts[b][:, s0:s0 + CH]),
            "wqkv": w_qkv,
            "wo": w_o,
            "cosw": np.ascontiguousarray(cos_t[:, s0:s0 + CH]),
            "sinw": np.ascontiguousarray(sin_t[:, s0:s0 + CH]),
            "masks": _masks_for_chunk(chunk),
            "perm": pm,
            "ident": identv,
            "vm": _vm_for_chunk(chunk),
        })
    return in_maps


def _install_ntff_shim():
    """bass_utils wants antenv.axon_hooks for trace=True under axon; this
    environment lacks that module, so synthesize it from the boot helper."""
    import sys
    import types
    if "antenv.axon_hooks" in sys.modules:
        return
    try:
        from trn_agent_boot.trn_boot import _ntff_profile_via_ctypes
        hook = _ntff_profile_via_ctypes("/opt/axon/libaxon_pjrt.so")
    except Exception:
        hook = None
    mod = types.ModuleType("antenv.axon_hooks")
    mod.get_axon_ntff_profile_hook = lambda: hook
    mod.set_axon_ntff_profile_hook = lambda h: None
    sys.modules["antenv.axon_hooks"] = mod


def run(x, w_qkv, w_o, trace=False):
    if "nc" not in _CACHE:
        _CACHE["nc"] = _build()
    nc = _CACHE["nc"]
    in_maps = _prepare_in_maps(np.asarray(x), np.asarray(w_qkv),
                               np.asarray(w_o))
    if trace:
        _install_ntff_shim()
    try:
        res = run_bass_kernel_spmd(nc, in_maps, list(range(NCORES)),
                                   trace=trace)
    except Exception:
        if not trace:
            raise
        res = run_bass_kernel_spmd(nc, in_maps, list(range(NCORES)),
                                   trace=False)
    y = np.empty((B, S, E), dtype=np.float32)
    for c in range(NCORES):
        b, chunk = divmod(c, 4)
        y[b, chunk * CH:(chunk + 1) * CH, :] = \
            np.asarray(res.results[c]["yt"], dtype=np.float32).T
    return y, res


def kernel(x, w_qkv, w_o):
    y, _ = run(x, w_qkv, w_o, trace=False)
    return y
